# revision 10
# baseline (speedup 1.0000x reference)
"""Causal multi-head attention (B=4, H=16, S=2048, D=64) on 8 TRN2 NeuronCores.

Sharding: B*H = 64 (batch, head) pairs -> 8 per core, fully independent,
no collectives.

Per-core algorithm (per head):
  - Q, K, V loaded with f32->bf16 cast DMAs (single SWDGE queue) into
    natural [128, 16, 64] block layout.
  - Q and K are transposed on the PE (16 identity-matmul block transposes
    per tensor into a [64, 1024] bf16 PSUM staging tile, batch-copied to
    SBUF by DVE; GpSimd cannot read PSUM) -> qt/kt [128, S] with [d, s] in rows 0:64. kt rows
    64:128 are zeroed once per ring slot (memset survives ring reuse);
    qt's bottom rows are zeroed once per ring slot too (junk would be
    harmless against zero weights only if finite; fresh SBUF may hold
    NaN bit patterns). No DRAM scratch, no XBAR transpose DMAs: the sync HWDGE
    ring carries only output stores, so nothing chains behind slow
    transpose storms on the shared physical DMA queues.
  - Scores per k-block kb: S^T[kb] = matmul(lhsT=kt[:, kb] (K=128, full
    rate), rhs=qt[:, q >= kb*128]) -> PSUM [128, <=1024], one
    exp(0.125 x) ScalarE activation per chunk -> U^T (bf16, unnormalized
    probs, transposed). Diagonal block masked by upper-triangular
    multiply (DVE).
  - PV per q-block: O[qb] = sum_kb U^T[kb].T @ [V[kb] | 1] accumulated in
    PSUM [128, 65]; col 64 is the softmax denominator. Normalize with
    per-partition reciprocal multiply, store bf16 via sync HWDGE
    (numpy converts to f32).

Pipelining: phase h interleaves, at k-block granularity: scores+exp of
head h, PE block-transposes for head h+1, and PV matmuls of head h-1 -
so ScalarE sees an uninterrupted stream of score chunks while the PE
fills its slack with transposes and PV.
"""

import numpy as np

import concourse.bass as bass
import concourse.tile as tile
from concourse import mybir
from concourse.bass_utils import run_bass_kernel_spmd
from concourse.masks import make_identity, make_upper_triangular
from concourse.vector_clock import ScopedClock, VectorClock

F32 = mybir.dt.float32
BF16 = mybir.dt.bfloat16

B, H, S, D = 4, 16, 2048, 64
N_CORES = 8
HEADS_PER_CORE = B * H // N_CORES  # 8
NB = S // 128  # 16 blocks of 128
SCALE = 1.0 / np.sqrt(np.float32(D))  # 0.125
CHUNK = 1024  # activation chunk (2 PSUM banks)


def _patch_tile_drain():
    """This walrus build rejects >1 sem wait on the kernel-tail Drain
    instruction ("Too many sync wait commands"). Spread the waits across
    single-wait NOPs on the sync engine instead."""
    if getattr(tile.TileContext, "_drain_patched", False):
        return

    def _drain_and_barrier(self, tick_clock, wait_clock):
        gc = tick_clock.global_clock
        n = len(gc)
        for i in range(n):
            if gc[i] > 0:
                vc = VectorClock([gc[j] if j == i else 0 for j in range(n)])
                nop_inst = self.nc.sync.nop(nofuse=True, hint=f"drainwait{i}")
                wait_clock.add_sem_waits(nop_inst.ins, ScopedClock({None: vc}))
        self.nc.sync.drain()
        self.nc.all_engine_barrier()
        popped = self.nc._tile_sem_poison_stack.pop()
        assert popped is self._sem_poison
        self.nc.clear_and_free_semaphores(list(self.sems.allocated().values()))
        self.nc.all_engine_barrier()

    tile.TileContext._drain_and_barrier = _drain_and_barrier
    tile.TileContext._drain_patched = True


_patch_tile_drain()


def _split_multi_waits(nc, limit=1):
    """This walrus build allows at most one sem wait per instruction.
    Move excess waits onto same-engine NOPs inserted just before."""
    ctr = [0]
    for func in nc.m.functions:
        for bb in func.blocks:
            insts = list(bb.instructions)
            out = []
            changed = False
            for inst in insts:
                si = inst.sync_info
                if si is not None and si.on_wait is not None and len(si.on_wait) > limit:
                    waits = list(si.on_wait)
                    extra, keep = waits[:-limit], waits[-limit:]
                    for w in extra:
                        ctr[0] += 1
                        nop = mybir.InstNoOp(
                            name=f"waitsplit-{ctr[0]}", ins=[], outs=[]
                        )
                        nop.engine = inst.engine
                        nop.sync_info = mybir.SyncInfo(on_wait=[w], on_update=[])
                        out.append(nop)
                    inst.sync_info = mybir.SyncInfo(
                        on_wait=keep, on_update=list(si.on_update or [])
                    )
                    changed = True
                out.append(inst)
            if changed:
                try:
                    bb.instructions[:] = out
                except Exception:
                    bb.instructions = out
    return nc


def build_nc(n_heads: int = HEADS_PER_CORE):
    nc = bass.Bass("TRN2", target_bir_lowering=False)
    q_d = nc.dram_tensor("queries", [n_heads, S, D], F32, kind="ExternalInput")
    k_d = nc.dram_tensor("keys", [n_heads, S, D], F32, kind="ExternalInput")
    v_d = nc.dram_tensor("values", [n_heads, S, D], F32, kind="ExternalInput")
    o_d = nc.dram_tensor("out", [n_heads, S, D], BF16, kind="ExternalOutput")

    # [h, p, n, d] views: s = n*128 + p
    q_r = q_d[:].rearrange("h (n p) d -> h p n d", p=128)
    k_r = k_d[:].rearrange("h (n p) d -> h p n d", p=128)
    v_r = v_d[:].rearrange("h (n p) d -> h p n d", p=128)
    o_r = o_d[:].rearrange("h (n p) d -> h p n d", p=128)

    KT_BUFS = 3

    with tile.TileContext(nc) as tc:
        with (
            tc.tile_pool(name="const", bufs=1) as constp,
            tc.tile_pool(name="nat", bufs=3) as natp,
            tc.tile_pool(name="tp", bufs=KT_BUFS) as tpp,
            tc.tile_pool(name="vpool", bufs=4) as vpp,
            tc.tile_pool(name="ut", bufs=3) as utp,
            tc.tile_pool(name="oh", bufs=3) as ohp,
            tc.tile_pool(name="rz", bufs=4) as rzp,
            tc.tile_pool(name="ps_s", bufs=3, space="PSUM") as ps_s,
            tc.tile_pool(name="ps_o", bufs=1, space="PSUM") as ps_o,
            tc.tile_pool(name="ps_t", bufs=1, space="PSUM") as ps_t,
        ):
            trimask = constp.tile([128, 128], BF16)
            make_upper_triangular(nc, trimask, val=1.0, diag=True)
            ident = constp.tile([128, 128], BF16)
            make_identity(nc, ident)

            nats = {}
            tts = {}
            vps = {}
            kt_zeroed = [0]

            def issue_casts(h):
                qn = natp.tile([128, NB, D], BF16, tag="qn")
                nc.gpsimd.dma_start(out=qn, in_=q_r[h])
                kn = natp.tile([128, NB, D], BF16, tag="kn")
                nc.gpsimd.dma_start(out=kn, in_=k_r[h])
                nats[h] = (qn, kn)
                vp = vpp.tile([128, NB, D + 1], BF16, tag="vp")
                nc.gpsimd.dma_start(out=vp[:, :, 0:D], in_=v_r[h])
                nc.vector.memset(vp[:, :, D : D + 1], 1.0)
                vps[h] = vp

            def alloc_tt(h):
                qt = tpp.tile([128, S], BF16, tag="qt")
                kt = tpp.tile([128, S], BF16, tag="kt")
                if kt_zeroed[0] < KT_BUFS:
                    kt_zeroed[0] += 1
                    nc.vector.memset(kt[64:128, :], 0.0)
                    # uninitialized SBUF can hold NaN bit patterns, and
                    # NaN * 0-weight is still NaN - zero qt's junk rows once
                    nc.vector.memset(qt[64:128, :], 0.0)
                tts[h] = (qt, kt)

            def emit_transpose_fill(h, fi):
                """fi 0/1: Q blocks 0-7 / 8-15; fi 2/3: same for K."""
                qn, kn = nats[h]
                qt, kt = tts[h]
                src, dst = (qn, qt) if fi < 2 else (kn, kt)
                base = (fi % 2) * 8
                pt = ps_t.tile([64, 8 * 128], BF16, tag="pt")
                for j in range(8):
                    nc.tensor.transpose(
                        pt[0:64, j * 128 : (j + 1) * 128],
                        src[:, base + j, :],
                        ident,
                    )
                nc.vector.tensor_copy(
                    out=dst[0:64, base * 128 : (base + 8) * 128], in_=pt
                )
                if fi == 3:
                    nats.pop(h)

            def emit_pv_qb(st, qb):
                uts, vp, oh = st["uts"], st["vp"], st["oh"]
                po = ps_o.tile([128, D + 1], F32, tag="o")
                for kb in range(qb + 1):
                    nc.tensor.matmul(
                        po,
                        lhsT=uts[kb][:, (qb - kb) * 128 : (qb - kb) * 128 + 128],
                        rhs=vp[:, kb, :],
                        start=(kb == 0),
                        stop=(kb == qb),
                    )
                rz = rzp.tile([128, 1], F32, tag="rz")
                nc.vector.reciprocal(rz, po[:, D : D + 1])
                nc.vector.tensor_scalar_mul(oh[:, qb, :], po[:, 0:D], rz)

            # startup: casts for heads 0-1, then head 0's transposes upfront
            issue_casts(0)
            issue_casts(1)
            alloc_tt(0)
            for fi in (0, 2, 1, 3):
                emit_transpose_fill(0, fi)

            prev = None
            for h in range(n_heads):
                if h + 2 < n_heads:
                    issue_casts(h + 2)
                if h + 1 < n_heads:
                    alloc_tt(h + 1)
                qt, kt = tts[h]
                vp = vps.pop(h)

                uts = []
                oh = ohp.tile([128, NB, D], BF16, tag="oh")
                cur = {"uts": uts, "vp": vp, "oh": oh, "h": h}
                for kb in range(NB):
                    L = S - kb * 128  # valid q length (q >= kb*128)
                    ut = utp.tile([128, L], BF16, tag=f"ut{kb}")
                    uts.append(ut)
                    off = 0
                    while off < L:
                        tl = min(CHUNK, L - off)
                        ps = ps_s.tile([128, CHUNK], F32, tag="s")
                        for c0 in range(0, tl, 512):
                            cl = min(512, tl - c0)
                            q0 = kb * 128 + off + c0
                            nc.tensor.matmul(
                                ps[:, c0 : c0 + cl],
                                lhsT=kt[:, kb * 128 : (kb + 1) * 128],
                                rhs=qt[:, q0 : q0 + cl],
                                start=True,
                                stop=True,
                            )
                        nc.scalar.activation(
                            out=ut[:, off : off + tl],
                            in_=ps[:, 0:tl],
                            func=mybir.ActivationFunctionType.Exp,
                            scale=float(SCALE),
                        )
                        off += tl
                    # mask diagonal block: keep k <= q (partition <= free)
                    nc.vector.tensor_mul(ut[:, 0:128], ut[:, 0:128], trimask)
                    # next head's PE block-transposes, late in the phase so
                    # its casts have certainly landed
                    if kb in (8, 10, 12, 14) and h + 1 < n_heads:
                        emit_transpose_fill(h + 1, (kb - 8) // 2)
                    # previous head's PV so ScalarE never idles
                    if prev is not None:
                        emit_pv_qb(prev, kb)
                    # last head has no successor phase: interleave its own
                    # PV at a 2-block lag so the tail drain is short
                    if h == n_heads - 1 and kb >= 2:
                        emit_pv_qb(cur, kb - 2)

                if prev is not None:
                    nc.sync.dma_start(out=o_r[prev["h"]], in_=prev["oh"])
                prev = cur

            for qb in range(NB - 2, NB):
                emit_pv_qb(prev, qb)
            nc.sync.dma_start(out=o_r[prev["h"]], in_=prev["oh"])
    _split_multi_waits(nc)
    return nc


_NC_CACHE = {}


def _get_nc(n_heads: int = HEADS_PER_CORE):
    if n_heads not in _NC_CACHE:
        _NC_CACHE[n_heads] = build_nc(n_heads)
    return _NC_CACHE[n_heads]


def make_in_maps(queries, keys, values):
    qf = np.ascontiguousarray(
        np.asarray(queries, dtype=np.float32).reshape(B * H, S, D)
    )
    kf = np.ascontiguousarray(np.asarray(keys, dtype=np.float32).reshape(B * H, S, D))
    vf = np.ascontiguousarray(
        np.asarray(values, dtype=np.float32).reshape(B * H, S, D)
    )
    n = HEADS_PER_CORE
    return [
        {
            "queries": qf[i * n : (i + 1) * n],
            "keys": kf[i * n : (i + 1) * n],
            "values": vf[i * n : (i + 1) * n],
        }
        for i in range(N_CORES)
    ]


def kernel(keys, queries, values, head_dim=None, **_ignored):
    nc = _get_nc()
    in_maps = make_in_maps(queries, keys, values)
    res = run_bass_kernel_spmd(nc, in_maps, core_ids=list(range(N_CORES)))
    out = np.concatenate([res.results[i]["out"] for i in range(N_CORES)], axis=0)
    return out.reshape(B, H, S, D).astype(np.float32)
